# revision 49
# baseline (speedup 1.0000x reference)
"""Trainium2 Bass kernel for the 21-joint hand-graph message-passing MLP.

Math (per sample b, per target joint t with neighbor list S_t of length n):
    g   = concat(x[b, S_t[0]], ..., x[b, S_t[n-1]])          # [n*64]
    h1  = relu(g @ W1_t + b1_t)                              # [128]
    h2  = relu(h1 @ W2_t + b2_t)                             # [128]
    out[b, t] = h2 @ W3_t + b3_t                             # [64]

Strategy (pure data parallel over 8 NeuronCores, B=65536 -> 8192/core):
  - x is pre-transposed ON THE HOST into a feature-major pair-tile layout
    xpk [15*128, BC] bf16: block p holds the 64 features of node u_p on
    partitions 0-63 and node v_p on 64-127.  The kernel reads it with ONE
    plain 3.84MB DMA per 1024-batch tile (sync HWDGE ring) -- no xbar
    transposes at all.  Host-side pair packing makes ARBITRARY node pairs
    available, so L1 runs the optimal per-target pairing: 28 genuine
    K=128 pair chunks + 13 zero-padded singles = 41 chunks (vs 47 for
    the adjacent-pair tile set), all full 128-row matmuls (partial-row
    matmuls that mix row groups in a PSUM bank can fault the device).
  - EVERY matmul is emitted as two concurrent (128,64) col-tile halves
    (mm2): uniform tiling mode means the PE never pays a mode-switch
    drain, T1 overlaps T0 (~4ns), and the L3 target pair tlo/thi runs
    genuinely concurrently (tlo on T0, thi on T1, distinct rhs streams).
  - L2 is weight-stationary (w2 [128,128]); relu+bias fused into the PSUM
    evacuations, load-balanced between ScalarE and VectorE with the
    HW-measured op costs.
  - L3: two adjacent targets share one 2-bank PSUM tile; b3 is a
    per-partition bias folded into the single [128,1024] evacuation,
    which writes bf16.  Stores go out on the gpsimd SWDGE queues
    (separate sem space from the HWDGE load ring); only the kernel's
    very last stores use HWDGE so the final fence walk ends sooner.
    The feature-major [1344, BC] bf16 result is transposed back to
    [B, 21, 64] f32 on the host.
  - The PE stream is software-pipelined (L1(k) | L2(k-3) | L3(k-5)),
    continuous across batch-tile boundaries; warm-up matmuls on a dummy
    tile keep the PE busy (and the HAM clock-gate open) during the
    initial weight/slab loads; one PSUM pool (4 x [128,1024], 8 banks)
    rotates psum1/psum2/psum3 allocations.
"""

import numpy as np
import ml_dtypes

B, J, D, H1, H2 = 65536, 21, 64, 128, 128
NCORES = 8
BC = B // NCORES          # 8192 samples per core
TILE = 1024               # batch tile (psum1/psum2 = 2 PSUM banks in fp32)
NTILES = BC // TILE       # 8
TPAIRS = 11               # L3 target pairs: (0,1),(2,3),...,(18,19),(20,)

# Host-packed x pair tiles: ANY (u, v) node pair can be a tile because the
# host lays them out adjacently.  Chosen so every target's neighbor list
# splits into ceil(n/2) chunks (28 pairs + 13 singles = 41 chunks).
XTILES = [(0, 13), (1, 5), (9, 17), (0, 5), (1, 2), (3, 4), (5, 6),
          (0, 1), (7, 8), (9, 10), (11, 12), (13, 14), (15, 16),
          (17, 18), (19, 20)]
NPAIRS = len(XTILES)
PAIR_TILE = {p: i for i, p in enumerate(XTILES)}
TILE_OF = {}              # node -> (tile_idx, half), first occurrence
for _i, (_u, _v) in enumerate(XTILES):
    TILE_OF.setdefault(_u, (_i, 0))
    TILE_OF.setdefault(_v, (_i, 1))

FINGER_BASE = [4 * f + 1 for f in range(5)]
NEIGH = {
    6: [[0, 1, 5, 9, 13, 17]],
    5: [[0, 5, 6, 1, 9], [0, 9, 10, 5, 13], [0, 13, 14, 9, 17]],
    4: [[0, 1, 2, 5], [0, 17, 18, 13]],
    3: [r for b in FINGER_BASE for r in ([b, b + 1, b + 2], [b + 1, b + 2, b + 3])],
    2: [[b + 2, b + 3] for b in FINGER_BASE],
}
OUT = {
    6: [0],
    5: [5, 9, 13],
    4: [1, 17],
    3: [j for b in FINGER_BASE for j in (b + 1, b + 2)],
    2: [b + 3 for b in FINGER_BASE],
}
GROUPS = [6, 5, 4, 3, 2]

# target t -> (group n, row index within its group, neighbor list)
TARGET = {}
for n in GROUPS:
    for row, t in enumerate(OUT[n]):
        TARGET[t] = (n, row, list(NEIGH[n][row]))

# Hand-chosen optimal pairing per target (node pairs; all are in XTILES).
PAIRING = {
    0: [(0, 13), (1, 5), (9, 17)],
    1: [(0, 5), (1, 2)],
    2: [(1, 2)],
    3: [(3, 4)],
    4: [(3, 4)],
    5: [(5, 6), (0, 1)],
    6: [(5, 6)],
    7: [(7, 8)],
    8: [(7, 8)],
    9: [(9, 10), (0, 13)],
    10: [(9, 10)],
    11: [(11, 12)],
    12: [(11, 12)],
    13: [(13, 14), (9, 17)],
    14: [(13, 14)],
    15: [(15, 16)],
    16: [(15, 16)],
    17: [(17, 18), (0, 13)],
    18: [(17, 18)],
    19: [(19, 20)],
    20: [(19, 20)],
}


def build_l1_plan():
    plan = {}
    for t in range(21):
        n, _, S = TARGET[t]
        used = [False] * n
        pairs = []
        for (u, v) in PAIRING[t]:
            i, k = S.index(u), S.index(v)
            assert not used[i] and not used[k]
            used[i] = used[k] = True
            pairs.append(dict(tile=PAIR_TILE[(u, v)], pos0=i, pos1=k))
        singles = []
        for i in range(n):
            if not used[i]:
                tile_idx, half = TILE_OF[S[i]]
                singles.append(dict(tile=tile_idx, pos=i, half=half))
        assert len(singles) <= 1
        plan[t] = dict(pairs=pairs, singles=singles)
    return plan


L1_PLAN = build_l1_plan()


def assign_w1_cols():
    cols = {}
    col = 0
    for t in range(21):
        p = L1_PLAN[t]
        for i, _ in enumerate(p["pairs"]):
            cols[(t, "pair", i)] = col
            col += 128
        for i, _ in enumerate(p["singles"]):
            cols[(t, "single", i)] = col
            col += 128
    return cols, col


W1_COLS, W1_NCOL = assign_w1_cols()      # 41 * 128 = 5248


def pack_weights(inputs):
    """Host-side prep: permute/pack all weights into flat bf16/f32 arrays."""
    bf16 = ml_dtypes.bfloat16
    w1p = np.zeros((128, W1_NCOL), np.float32)
    for t in range(21):
        n, row, S = TARGET[t]
        W1 = np.asarray(inputs[f"w1_g{n}"][row], np.float32)  # [n*64, 128]
        p = L1_PLAN[t]
        for i, pr in enumerate(p["pairs"]):
            c = W1_COLS[(t, "pair", i)]
            w1p[0:64, c:c + 128] = W1[64 * pr["pos0"]:64 * pr["pos0"] + 64]
            w1p[64:128, c:c + 128] = W1[64 * pr["pos1"]:64 * pr["pos1"] + 64]
        for i, e in enumerate(p["singles"]):
            c = W1_COLS[(t, "single", i)]
            half = e["half"]
            w1p[64 * half:64 * half + 64, c:c + 128] = \
                W1[64 * e["pos"]:64 * e["pos"] + 64]
    w2p = np.zeros((128, 128 * 21), np.float32)
    w3p = np.zeros((128, 64 * 21), np.float32)
    b1p = np.zeros((128, 21), np.float32)
    b2p = np.zeros((128, 21), np.float32)
    b3p = np.zeros((128, TPAIRS), np.float32)   # per-partition bias, paired
    for t in range(21):
        n, row, _ = TARGET[t]
        w2p[:, 128 * t:128 * (t + 1)] = np.asarray(inputs[f"w2_g{n}"][row])
        w3p[:, 64 * t:64 * (t + 1)] = np.asarray(inputs[f"w3_g{n}"][row])
        b1p[:, t] = np.asarray(inputs[f"b1_g{n}"][row])
        b2p[:, t] = np.asarray(inputs[f"b2_g{n}"][row])
        b3p[64 * (t % 2):64 * (t % 2) + 64, t // 2] = \
            np.asarray(inputs[f"b3_g{n}"][row])
    return dict(
        w1p=w1p.astype(bf16), w2p=w2p.astype(bf16), w3p=w3p.astype(bf16),
        b1p=b1p, b2p=b2p, b3p=b3p,
    )


def pack_x(x):
    """x [Bn, 21, 64] f32 -> xpk [NPAIRS*128, Bn] bf16 (feature-major pairs)."""
    bf16 = ml_dtypes.bfloat16
    Bn = x.shape[0]
    xt = np.ascontiguousarray(
        np.asarray(x, np.float32).astype(bf16).transpose(1, 2, 0))  # [21,64,Bn]
    xpk = np.empty((NPAIRS * 128, Bn), bf16)
    for p, (u, v) in enumerate(XTILES):
        xpk[128 * p:128 * p + 64] = xt[u]
        xpk[128 * p + 64:128 * p + 128] = xt[v]
    return xpk


def numpy_emulate(inputs, x):
    """Bit-layout-faithful numpy model of the HW kernel (minus PSUM rounding):
    validates the chunk plan / packing / L3 pairing offline."""
    bf16 = ml_dtypes.bfloat16
    packed = pack_weights(inputs)
    xpk = pack_x(x)
    Bn = x.shape[0]
    out = np.zeros((Bn, 21, 64), np.float32)
    for t in range(21):
        psum1 = np.zeros((128, Bn), np.float32)
        pl = L1_PLAN[t]
        chunks = [("pair", i, pr["tile"]) for i, pr in enumerate(pl["pairs"])]
        chunks += [("single", i, e["tile"]) for i, e in enumerate(pl["singles"])]
        for kind, i, tl in chunks:
            c = W1_COLS[(t, kind, i)]
            lhsT = packed["w1p"][:, c:c + 128].astype(np.float32)
            rhs = xpk[128 * tl:128 * tl + 128].astype(np.float32)
            psum1 += lhsT.T @ rhs
        h1 = np.maximum(psum1 + packed["b1p"][:, t:t + 1], 0).astype(bf16)
        w2 = packed["w2p"][:, 128 * t:128 * (t + 1)].astype(np.float32)
        psum2 = w2.T @ h1.astype(np.float32)
        h2 = np.maximum(psum2 + packed["b2p"][:, t:t + 1], 0).astype(bf16)
        w3 = packed["w3p"][:, 64 * t:64 * (t + 1)].astype(np.float32)
        b3 = packed["b3p"][64 * (t % 2):64 * (t % 2) + 64, t // 2]
        o = (w3.T @ h2.astype(np.float32) + b3[:, None]).astype(bf16)
        out[:, t] = o.T.astype(np.float32)
    return out


# ---------------------------------------------------------------------------
# Bass kernel
# ---------------------------------------------------------------------------

def build_bass_kernel(num_devices=NCORES, bc=BC):
    import concourse.bass as bass
    import concourse.tile as tile
    from concourse import bacc, mybir

    bf16 = mybir.dt.bfloat16
    f32 = mybir.dt.float32
    Relu = mybir.ActivationFunctionType.Relu
    Ident = mybir.ActivationFunctionType.Identity
    Alu = mybir.AluOpType
    ntiles = bc // TILE

    nc = bacc.Bacc("TRN2", target_bir_lowering=False, debug=False,
                   num_devices=num_devices)
    x_dram = nc.dram_tensor("xpk", [NPAIRS * 128, bc], bf16,
                            kind="ExternalInput").ap()
    out_dram = nc.dram_tensor("outf", [J * D, bc], bf16,
                              kind="ExternalOutput").ap()
    w1_dram = nc.dram_tensor("w1p", [128, W1_NCOL], bf16, kind="ExternalInput").ap()
    w2_dram = nc.dram_tensor("w2p", [128, 128 * 21], bf16, kind="ExternalInput").ap()
    w3_dram = nc.dram_tensor("w3p", [128, 64 * 21], bf16, kind="ExternalInput").ap()
    b1_dram = nc.dram_tensor("b1p", [128, 21], f32, kind="ExternalInput").ap()
    b2_dram = nc.dram_tensor("b2p", [128, 21], f32, kind="ExternalInput").ap()
    b3_dram = nc.dram_tensor("b3p", [128, TPAIRS], f32, kind="ExternalInput").ap()
    # [128, NPAIRS, bc] view for the one-DMA-per-batch-tile slab load
    x_view = x_dram.rearrange("(p r) b -> r p b", p=NPAIRS, r=128)

    with tile.TileContext(nc) as tc:
        with (
            tc.tile_pool(name="wpool", bufs=1) as wpool,
            tc.tile_pool(name="xtp", bufs=4) as xtp,
            tc.tile_pool(name="h1p", bufs=4) as h1p,
            tc.tile_pool(name="h2p", bufs=1) as h2p,
            tc.tile_pool(name="outp", bufs=6) as outp,
            tc.tile_pool(name="psp", bufs=4, space="PSUM") as psp,
        ):
            w1s = wpool.tile([128, W1_NCOL], bf16, name="w1s")
            w2s = wpool.tile([128, 128 * 21], bf16, name="w2s")
            w3s = wpool.tile([128, 64 * 21], bf16, name="w3s")
            b1s = wpool.tile([128, 21], f32, name="b1s")
            b2s = wpool.tile([128, 21], f32, name="b2s")
            b3s = wpool.tile([128, TPAIRS], f32, name="b3s")
            dummy = wpool.tile([128, 648], bf16, name="dummy")

            # greedy ScalarE/VectorE balance for the PSUM evacuations,
            # using HW-measured op costs (ACT 1113ns / DVE 1284ns @ FD=1024)
            ev_time = [0.0, 0.0]          # ns on [ACT, DVE]
            ev_n = [0]

            def evac(dst, src, bias, relu, fd):
                act_cost = (312.0 + fd) / 1.2
                dve_cost = (210.0 + fd * 1.05) / 0.96
                ev_n[0] += 1
                # the first evacuations go to the idle VectorE: the ACT
                # queue is still draining the head's weight-DMA triggers,
                # and a delayed first h1 evac stalls the PSUM rotation
                if ev_n[0] > 3 and \
                        ev_time[0] + act_cost <= ev_time[1] + dve_cost:
                    ev_time[0] += act_cost
                    nc.scalar.activation(dst, src, Relu if relu else Ident,
                                         bias=bias, scale=1.0)
                else:
                    ev_time[1] += dve_cost
                    if relu:
                        nc.vector.tensor_scalar(dst, src, bias, 0.0,
                                                Alu.add, Alu.max)
                    else:
                        nc.vector.tensor_scalar(dst, src, bias, None, Alu.add)

            def mm2(out_ap, wcol_lo, wcol_hi, wtile, rhs_lo, rhs_hi,
                    start, stop, base=0):
                """One N=512 slot as two concurrent 64-col tile matmuls.
                All matmuls in the kernel use (128,64) tiling so the PE
                never switches tiling mode (mode switches drain the array
                and serialize; uniform mode lets T0/T1 run concurrently)."""
                nc.tensor.matmul(out_ap[base:base + 64, :], wtile[:, wcol_lo],
                                 rhs_lo, start=start, stop=stop,
                                 skip_group_check=True)
                nc.tensor.matmul(out_ap[base + 64:base + 128, :],
                                 wtile[:, wcol_hi],
                                 rhs_hi, start=start, stop=stop,
                                 skip_group_check=True)

            def slab_tile():
                slab = xtp.tile([128, NPAIRS * TILE], bf16, tag="slab",
                                name="slab")
                slab3 = slab[:].rearrange("r (p b) -> r p b", p=NPAIRS, b=TILE)
                return slab, slab3

            def issue_load(it):
                """ONE big DMA per batch-tile: a burst of small DMAs
                backlogs the HWDGE ring and the framework's DMA-semaphore
                reuse fences in the PE queue then stall on them.  The
                single DMA is issued two tiles ahead, so any fence on it
                is long-satisfied."""
                b0 = it * TILE
                slab, slab3 = slab_tile()
                nc.sync.dma_start(slab3, x_view[:, :, b0:b0 + TILE])
                return slab

            # Target order: L3 pairs (2a, 2a+1) must stay adjacent, but the
            # PAIR order is free.  For tiles >= 1, the lightest pair (t20,
            # one L1 chunk) is moved right after the heaviest (t0/t1, five
            # chunks) so the PE never sprints far ahead of the evacuation
            # engines (which exhausts the PSUM rotation and stalls).  Tile
            # 0 keeps 0..20 so its split loads arrive in first-use order.
            ORDER1 = [0, 1, 20] + list(range(2, 20))
            units = [(it, t) for it in range(ntiles)
                     for t in (range(21) if it == 0 else ORDER1)]
            NU = len(units)
            # per-pair slab loads run TWO batch-tiles ahead on the sync
            # HWDGE ring, in first-use order; w1 is split so the columns
            # for the first targets arrive first.
            # Tile-0 pair tiles load in first-use-order chunks on the sync
            # ring; weights ride the scalar ring in parallel.  w1 is split
            # so w1b is first needed at t13 (~26us in), after the HWDGE
            # sem-lane fence ahead of it has long released.
            W1SPLIT = 3200       # end of target 12's w1 columns
            slab0, slab0v = slab_tile()
            for p0, p1 in ((0, 2), (2, 4), (4, 6), (6, 9), (9, 12),
                           (12, NPAIRS)):
                nc.sync.dma_start(slab0v[:, p0:p1, :],
                                  x_view[:, p0:p1, 0:TILE])
            xts = {0: slab0}
            nc.scalar.dma_start(w1s[:, 0:W1SPLIT], w1_dram[:, 0:W1SPLIT])
            nc.scalar.dma_start(b1s[:], b1_dram)
            nc.scalar.dma_start(w2s[:], w2_dram)
            nc.scalar.dma_start(b2s[:], b2_dram)
            nc.scalar.dma_start(w3s[:], w3_dram)
            nc.scalar.dma_start(b3s[:], b3_dram)
            nc.scalar.dma_start(w1s[:, W1SPLIT:], w1_dram[:, W1SPLIT:])
            h1t = {}
            h2t = {}

            # PE warm-up: dummy (128,64)-tiled matmuls with no data deps
            # keep the PE busy through the initial load phase so the HAM
            # clock-gate is released before the first real matmul.
            nc.vector.memset(dummy[:], 0.0)
            # preload the ACT spline-table set (~2.7us one-time) during the
            # load phase so the first real h1 evacuation doesn't pay it.
            # Uses a column the warm-up matmuls never touch, so the PE
            # warm-up doesn't serialize behind the table load.
            nc.scalar.activation(dummy[:, 640:641], dummy[:, 640:641], Relu,
                                 scale=1.0)
            warm = psp.tile([128, TILE], f32, tag="ps", name="ps")
            for _ in range(12):
                mm2(warm[:, 0:512], slice(0, 64), slice(64, 128), dummy,
                    dummy[:, 128:640], dummy[:, 128:640], True, True)

            def stage_l1(k):
                it, t = units[k]
                # slab prefetch runs THREE batch-tiles ahead (4-deep pool):
                # every slab DMA then issues with its pool buffer already
                # free and completes a full tile before any DMA-semaphore
                # fence can reference it — the in-flight-slab fences froze
                # the evacuation queues for ~8.6us per tile otherwise.
                # Tiles 1-3 are staggered through tile 0 to keep the head's
                # fence window clear.
                if it == 0 and ntiles > 1:
                    if t == 2:
                        xts[1] = issue_load(1)
                    elif t == 8 and ntiles > 2:
                        xts[2] = issue_load(2)
                    elif t == 14 and ntiles > 3:
                        xts[3] = issue_load(3)
                if t == 0 and it >= 1 and it + 3 < ntiles:
                    xts[it + 3] = issue_load(it + 3)
                slab = xts[it]
                pl = L1_PLAN[t]
                psum1 = psp.tile([128, TILE], f32, tag="ps", name="ps")
                chunks = []
                for i, pr in enumerate(pl["pairs"]):
                    chunks.append((W1_COLS[(t, "pair", i)], pr["tile"]))
                for i, e in enumerate(pl["singles"]):
                    chunks.append((W1_COLS[(t, "single", i)], e["tile"]))
                nch = len(chunks)
                for ci, (c, tl) in enumerate(chunks):
                    for h in range(2):
                        rhs = slab[:, TILE * tl + 512 * h:TILE * tl + 512 * (h + 1)]
                        mm2(psum1[:, 512 * h:512 * (h + 1)],
                            slice(c, c + 64), slice(c + 64, c + 128), w1s,
                            rhs, rhs, ci == 0, ci == nch - 1)
                h1 = h1p.tile([128, TILE], bf16, tag="h1", name="h1")
                evac(h1[:], psum1[:], b1s[:, t:t + 1], True, TILE)
                h1t[k] = h1

            def stage_l2(k):
                it, t = units[k]
                h1 = h1t.pop(k)
                psum2 = psp.tile([128, TILE], f32, tag="ps", name="ps")
                for h in range(2):
                    rhs = h1[:, 512 * h:512 * (h + 1)]
                    mm2(psum2[:, 512 * h:512 * (h + 1)],
                        slice(128 * t, 128 * t + 64),
                        slice(128 * t + 64, 128 * (t + 1)), w2s,
                        rhs, rhs, True, True)
                h2 = h2p.tile([128, TILE], bf16, tag=f"h2_{k % 4}",
                              name=f"h2_{k % 4}")
                evac(h2[:], psum2[:], b2s[:, t:t + 1], True, TILE)
                h2t[k] = h2

            def stage_l3(k):
                it, t = units[k]
                if not (t % 2 == 1 or t == 20):
                    return
                b0 = it * TILE
                tp = t // 2
                if t % 2 == 1:
                    tlo, thi = t - 1, t
                    h2lo, h2hi = h2t.pop(k - 1), h2t.pop(k)
                    rows = 128
                else:
                    tlo, thi = t, None
                    h2lo, h2hi = h2t.pop(k), None
                    rows = 64
                ot = outp.tile([128, TILE], bf16, tag="ot", name="ot")
                psum3 = psp.tile([128, TILE], f32, tag="ps", name="ps")
                for h in range(2):
                    if thi is not None:
                        # tlo on col tile T0, thi on T1 — concurrent
                        mm2(psum3[:, 512 * h:512 * (h + 1)],
                            slice(64 * tlo, 64 * tlo + 64),
                            slice(64 * thi, 64 * thi + 64), w3s,
                            h2lo[:, 512 * h:512 * (h + 1)],
                            h2hi[:, 512 * h:512 * (h + 1)], True, True)
                    else:
                        nc.tensor.matmul(
                            psum3[0:64, 512 * h:512 * (h + 1)],
                            w3s[:, 64 * tlo:64 * tlo + 64],
                            h2lo[:, 512 * h:512 * (h + 1)],
                            start=True, stop=True, skip_group_check=True)
                evac(ot[0:rows, :], psum3[0:rows, :], b3s[0:rows, tp:tp + 1],
                     False, TILE)
                # only the kernel's very last stores go HWDGE (completes
                # faster than SWDGE, shortening the end-of-kernel fence
                # walk); earlier scalar-ring stores would delay ACT evacs.
                dma = nc.scalar.dma_start \
                    if (it == ntiles - 1 and t >= 17) else nc.gpsimd.dma_start
                dma(out_dram[128 * tp:128 * tp + rows, b0:b0 + TILE],
                    ot[0:rows, :])

            for k in range(NU + 5):
                if k < NU:
                    stage_l1(k)
                if 0 <= k - 3 < NU:
                    stage_l2(k - 3)
                if 0 <= k - 5 < NU:
                    stage_l3(k - 5)

    nc.compile()
    return nc


PACKED = None
_NC = None
LAST_RESULT = None


def prepare(inputs):
    """Build (once) the bass module and the per-core input maps."""
    global PACKED, _NC
    import sys
    if "/opt/trn_rl_repo" not in sys.path:
        sys.path.insert(0, "/opt/trn_rl_repo")
    x = np.asarray(inputs["x"], np.float32)
    PACKED = pack_weights(inputs)
    if _NC is None:
        _NC = build_bass_kernel()
    in_maps = []
    for core in range(NCORES):
        m = dict(PACKED)
        m["xpk"] = pack_x(x[core * BC:(core + 1) * BC])
        in_maps.append(m)
    return _NC, in_maps


def kernel(**inputs):
    global LAST_RESULT
    nc, in_maps = prepare(inputs)
    from concourse.bass_utils import run_bass_kernel_spmd
    res = run_bass_kernel_spmd(nc, in_maps, core_ids=list(range(NCORES)))
    LAST_RESULT = res
    # outf is [1344, BC] bf16 feature-major per core; unshard + transpose host-side.
    out = np.empty((B, J, D), np.float32)
    for core, r in enumerate(res.results):
        fm = np.asarray(r["outf"]).reshape(J, D, BC).astype(np.float32)
        out[core * BC:(core + 1) * BC] = fm.transpose(2, 0, 1)
    return out
